# Initial kernel scaffold
#
"""Sparse (tanh-clipped, key-masked) dot-product attention on 8 trn2 NeuronCores.

Reference computation (per batch b, head h):
    logits = (Q @ K^T) / 8
    logits = 10 * tanh(logits)
    logits[masked keys] = -inf          (mask is per (batch, key))
    out = softmax(logits) @ V

Strategy:
  - Host: gather K/V down to the unmasked keys (~50% of 2048), pad to a
    common multiple of 128 across batches.  Pre-transpose Q and K to
    [64, S] so the contraction dim (d=64) is on partitions.  Build
    Vaug = [V | ones] so the softmax denominator falls out of the same
    matmul that computes the numerator.  Pad keys get an exp() bias of
    -100 so they contribute exp(10*tanh + -100) ~= 0.
  - Device (per core: one batch, 8 heads): for each 128-key tile,
    PE: S_T[k, q] = Kt^T-tile @ Qt;  ACT: tanh(s/8) then exp(10*t + bias);
    PE: accumulate Vaug^T @ P_T -> [65, 2048] PSUM (row 64 = denominator).
    Epilogue: PE-transpose [65, 128q] chunks, DVE reciprocal + scale,
    DMA out in natural [q, d] layout.
  - softmax needs no max-subtraction: 10*tanh is bounded in [-10, 10].
"""

import sys

if "/opt/trn_rl_repo" not in sys.path:
    sys.path.insert(0, "/opt/trn_rl_repo")

import numpy as np

import concourse.bass as bass
import concourse.tile as tile
from concourse import mybir
from concourse.bass_utils import run_bass_kernel_spmd
from concourse.masks import make_identity

B, H, S, D = 4, 16, 2048, 64
N_CORES = 8
HPC = H * B // N_CORES // B  # heads per core = 8 (each core: 1 batch, 8 heads)
Q_CHUNK = 512  # fp32 matmul moving-operand / PSUM-bank limit
F32 = mybir.dt.float32

_kernel_cache = {}


def _build_kernel(n_kp: int):
    """Build the per-core Bass program for n_kp (padded) kept keys."""
    n_kt = n_kp // 128
    nc = bass.Bass()

    qt_p = nc.declare_dram_parameter("qt", [HPC, D, S], F32, isOutput=False)
    kt_p = nc.declare_dram_parameter("kt", [HPC, D, n_kp], F32, isOutput=False)
    v_p = nc.declare_dram_parameter("vaug", [HPC, 128, n_kt, D + 1], F32, isOutput=False)
    bias_p = nc.declare_dram_parameter("bias", [128, n_kt], F32, isOutput=False)
    out_p = nc.declare_dram_parameter("out", [HPC, S, D], F32, isOutput=True)

    n_qc = S // Q_CHUNK  # 4 q-chunks of 512
    with tile.TileContext(nc) as tc:
        with (
            tc.tile_pool(name="consts", bufs=1) as consts,
            tc.tile_pool(name="inq", bufs=2) as inq,
            tc.tile_pool(name="ink", bufs=2) as ink,
            tc.tile_pool(name="inv", bufs=2) as inv,
            tc.tile_pool(name="act", bufs=2) as act_pool,
            tc.tile_pool(name="probs", bufs=3) as probs_pool,
            tc.tile_pool(name="epi", bufs=2) as epi_pool,
            tc.tile_pool(name="outsb", bufs=4) as out_pool,
            tc.tile_pool(name="ps_st", bufs=2, space="PSUM") as ps_st,
            tc.tile_pool(name="ps_out", bufs=1, space="PSUM") as ps_out,
        ):
            identity = consts.tile([128, 128], F32)
            make_identity(nc, identity)
            bias_sb = consts.tile([128, n_kt], F32)
            nc.sync.dma_start(out=bias_sb, in_=bias_p[:])

            for h in range(HPC):
                qt_sb = inq.tile([D, S], F32, tag="qt")
                nc.sync.dma_start(out=qt_sb, in_=qt_p[h])
                kt_sb = ink.tile([D, n_kp], F32, tag="kt")
                nc.sync.dma_start(out=kt_sb, in_=kt_p[h])
                v_sb = inv.tile([128, n_kt, D + 1], F32, tag="v")
                nc.sync.dma_start(out=v_sb, in_=v_p[h])

                out_ps = ps_out.tile([D + 1, S], F32, tag="out")
                for t in range(n_kt):
                    # two PSUM tiles of [128, 1024] per key-tile (PSUM budget)
                    t_sb = act_pool.tile([128, S], F32, tag="tanh")
                    for half in range(2):
                        st_ps = ps_st.tile([128, 2 * Q_CHUNK], F32, tag="st")
                        for i in range(2):
                            qc = 2 * half + i
                            nc.tensor.matmul(
                                st_ps[:, i * Q_CHUNK : (i + 1) * Q_CHUNK],
                                lhsT=kt_sb[:, t * 128 : (t + 1) * 128],
                                rhs=qt_sb[:, qc * Q_CHUNK : (qc + 1) * Q_CHUNK],
                                start=True,
                                stop=True,
                            )
                        nc.scalar.activation(
                            t_sb[:, half * 1024 : (half + 1) * 1024],
                            st_ps,
                            mybir.ActivationFunctionType.Tanh,
                            scale=0.125,
                        )
                    p_sb = probs_pool.tile([128, S], F32, tag="p")
                    nc.scalar.activation(
                        p_sb,
                        t_sb,
                        mybir.ActivationFunctionType.Exp,
                        bias=bias_sb[:, t : t + 1],
                        scale=10.0,
                    )
                    for qc in range(n_qc):
                        nc.tensor.matmul(
                            out_ps[:, qc * Q_CHUNK : (qc + 1) * Q_CHUNK],
                            lhsT=v_sb[:, t, :],
                            rhs=p_sb[:, qc * Q_CHUNK : (qc + 1) * Q_CHUNK],
                            start=(t == 0),
                            stop=(t == n_kt - 1),
                        )

                # epilogue: transpose back to [q, d], normalize, store
                o_sb = epi_pool.tile([D + 1, S], F32, tag="osb")
                nc.vector.tensor_copy(o_sb, out_ps)
                for qi in range(S // 128):
                    tr_ps = ps_st.tile([128, D + 1], F32, tag="st")
                    nc.tensor.transpose(
                        tr_ps,
                        o_sb[:, qi * 128 : (qi + 1) * 128],
                        identity[: D + 1, : D + 1],
                    )
                    recip = out_pool.tile([128, 1], F32, tag="recip")
                    nc.vector.reciprocal(recip, tr_ps[:, D : D + 1])
                    oq = out_pool.tile([128, D], F32, tag="oq")
                    nc.vector.tensor_scalar_mul(oq, tr_ps[:, 0:D], recip)
                    nc.sync.dma_start(
                        out=out_p[h, qi * 128 : (qi + 1) * 128, :], in_=oq
                    )
    return nc


def _prep_inputs(q, k, v, mask):
    """Host-side shard + gather + layout. Returns (in_maps, n_kp)."""
    keep = [np.flatnonzero(~mask[b, :, 0]) for b in range(B)]
    n_kp = max(128, -(-max(len(kb) for kb in keep) // 128) * 128)
    n_kt = n_kp // 128

    in_maps = []
    for c in range(N_CORES):
        b = c // 2
        h0 = (c % 2) * HPC
        kb = keep[b]
        nk = len(kb)

        qt = np.ascontiguousarray(q[b, h0 : h0 + HPC].transpose(0, 2, 1))

        kg = np.zeros((HPC, n_kp, D), np.float32)
        kg[:, :nk] = k[b, h0 : h0 + HPC][:, kb]
        kt = np.ascontiguousarray(kg.transpose(0, 2, 1))

        vg = np.zeros((HPC, n_kp, D + 1), np.float32)
        vg[:, :nk, :D] = v[b, h0 : h0 + HPC][:, kb]
        vg[:, :, D] = 1.0
        # [HPC, n_kt, 128, 65] -> [HPC, 128, n_kt, 65] (partition-major)
        vaug = np.ascontiguousarray(
            vg.reshape(HPC, n_kt, 128, D + 1).transpose(0, 2, 1, 3)
        )

        bias = np.zeros((128, n_kt), np.float32)
        idx = np.arange(n_kp).reshape(n_kt, 128).T  # [128, n_kt]
        bias[idx >= nk] = -100.0

        in_maps.append({"qt": qt, "kt": kt, "vaug": vaug, "bias": bias})
    return in_maps, n_kp


def kernel(q, k, v, mask, _trace=False):
    q = np.asarray(q, np.float32)
    k = np.asarray(k, np.float32)
    v = np.asarray(v, np.float32)
    mask = np.asarray(mask, bool)

    in_maps, n_kp = _prep_inputs(q, k, v, mask)
    if n_kp not in _kernel_cache:
        _kernel_cache[n_kp] = _build_kernel(n_kp)
    nc = _kernel_cache[n_kp]

    res = run_bass_kernel_spmd(nc, in_maps, list(range(N_CORES)), trace=_trace)
    out = np.empty((B, H, S, D), np.float32)
    for c in range(N_CORES):
        b = c // 2
        h0 = (c % 2) * HPC
        out[b, h0 : h0 + HPC] = res.results[c]["out"]
    if _trace:
        return out, res
    return out


if __name__ == "__main__":
    rng = np.random.default_rng(0)
    q = rng.standard_normal((B, H, S, D), np.float32)
    k = rng.standard_normal((B, H, S, D), np.float32)
    v = rng.standard_normal((B, H, S, D), np.float32)
    mask = rng.integers(0, 2, (B, S, 1)).astype(bool)
    out = kernel(q, k, v, mask)
    print("out", out.shape, out.dtype, float(np.abs(out).max()))


# revision 7
# speedup vs baseline: 2.2215x; 2.2215x over previous
"""Sparse (tanh-clipped, key-masked) dot-product attention on 8 trn2 NeuronCores.

Reference computation (per batch b, head h):
    logits = (Q @ K^T) / 8
    logits = 10 * tanh(logits)
    logits[masked keys] = -inf          (mask is per (batch, key))
    out = softmax(logits) @ V

Strategy:
  - Host: gather K/V down to the unmasked keys (~50% of 2048), pad to a
    common multiple of 128 across batches.  Pre-transpose Q and K to
    [64, S] so the contraction dim (d=64) is on partitions.  Build
    Vaug = [V | ones] so the softmax denominator falls out of the same
    matmul that computes the numerator.  Pad keys get an exp() bias of
    -100 so they contribute exp(10*tanh + -100) ~= 0.
  - Device (per core: one batch, 8 heads): for each 128-key tile,
    PE: S_T[k, q] = Kt^T-tile @ Qt;  ACT: tanh(s/8) then exp(10*t + bias);
    PE: accumulate Vaug^T @ P_T -> [65, 2048] PSUM (row 64 = denominator).
    Epilogue: PE-transpose [65, 128q] chunks, DVE reciprocal + scale,
    DMA out in natural [q, d] layout.
  - softmax needs no max-subtraction: 10*tanh is bounded in [-10, 10].
"""

import sys

if "/opt/trn_rl_repo" not in sys.path:
    sys.path.insert(0, "/opt/trn_rl_repo")

import numpy as np

import concourse.bass as bass
import concourse.tile as tile
from concourse import bacc, mybir
from concourse.bass_utils import run_bass_kernel_spmd
from concourse.masks import make_identity

B, H, S, D = 4, 16, 2048, 64
N_CORES = 8
HPC = B * H // N_CORES  # heads per core = 8 (each core: 1 batch, 8 heads)
Q_CHUNK = 512  # fp32 matmul moving-operand / PSUM-bank limit
F32 = mybir.dt.float32

_kernel_cache = {}


def _build_kernel(n_kp: int, reps: int = 1):
    """Build the per-core Bass program for n_kp (padded) kept keys.

    reps > 1 repeats the whole computation (for overhead-free timing).
    """
    n_kt = n_kp // 128
    nc = bacc.Bacc(None)

    qt_p = nc.declare_dram_parameter("qt", [HPC, D, S], F32, isOutput=False)
    kt_p = nc.declare_dram_parameter("kt", [HPC, D, n_kp], F32, isOutput=False)
    v_p = nc.declare_dram_parameter("vaug", [HPC, 128, n_kt, D + 1], F32, isOutput=False)
    bias_p = nc.declare_dram_parameter("bias", [128, n_kt], F32, isOutput=False)
    out_p = nc.declare_dram_parameter("out", [HPC, S, D], F32, isOutput=True)

    n_qc = S // Q_CHUNK  # 4 q-chunks of 512
    with tile.TileContext(nc) as tc:
        with (
            tc.tile_pool(name="consts", bufs=1) as consts,
            tc.tile_pool(name="inq", bufs=2) as inq,
            tc.tile_pool(name="ink", bufs=2) as ink,
            tc.tile_pool(name="inv", bufs=2) as inv,
            tc.tile_pool(name="act", bufs=2) as act_pool,
            tc.tile_pool(name="probs", bufs=3) as probs_pool,
            tc.tile_pool(name="epi", bufs=2) as epi_pool,
            tc.tile_pool(name="outsb", bufs=4) as out_pool,
            tc.tile_pool(name="ps_st", bufs=2, space="PSUM") as ps_st,
            tc.tile_pool(name="ps_out", bufs=1, space="PSUM") as ps_out,
        ):
            identity = consts.tile([128, 128], F32)
            make_identity(nc, identity)
            bias_sb = consts.tile([128, n_kt], F32)
            nc.sync.dma_start(out=bias_sb, in_=bias_p[:])

            for h in [h for _ in range(reps) for h in range(HPC)]:
                qt_sb = inq.tile([D, S], F32, tag="qt")
                nc.sync.dma_start(out=qt_sb, in_=qt_p[h])
                kt_sb = ink.tile([D, n_kp], F32, tag="kt")
                nc.sync.dma_start(out=kt_sb, in_=kt_p[h])
                v_sb = inv.tile([128, n_kt, D + 1], F32, tag="v")
                nc.sync.dma_start(out=v_sb, in_=v_p[h])

                out_ps = ps_out.tile([D + 1, S], F32, tag="out")
                for t in range(n_kt):
                    # two PSUM tiles of [128, 1024] per key-tile (PSUM budget)
                    t_sb = act_pool.tile([128, S], F32, tag="tanh")
                    for half in range(2):
                        st_ps = ps_st.tile([128, 2 * Q_CHUNK], F32, tag="st")
                        for i in range(2):
                            qc = 2 * half + i
                            nc.tensor.matmul(
                                st_ps[:, i * Q_CHUNK : (i + 1) * Q_CHUNK],
                                lhsT=kt_sb[:, t * 128 : (t + 1) * 128],
                                rhs=qt_sb[:, qc * Q_CHUNK : (qc + 1) * Q_CHUNK],
                                start=True,
                                stop=True,
                            )
                        nc.scalar.activation(
                            t_sb[:, half * 1024 : (half + 1) * 1024],
                            st_ps,
                            mybir.ActivationFunctionType.Tanh,
                            scale=0.125,
                        )
                    p_sb = probs_pool.tile([128, S], F32, tag="p")
                    nc.scalar.activation(
                        p_sb,
                        t_sb,
                        mybir.ActivationFunctionType.Exp,
                        bias=bias_sb[:, t : t + 1],
                        scale=10.0,
                    )
                    for qc in range(n_qc):
                        nc.tensor.matmul(
                            out_ps[:, qc * Q_CHUNK : (qc + 1) * Q_CHUNK],
                            lhsT=v_sb[:, t, :],
                            rhs=p_sb[:, qc * Q_CHUNK : (qc + 1) * Q_CHUNK],
                            start=(t == 0),
                            stop=(t == n_kt - 1),
                        )

                # epilogue: transpose back to [q, d], normalize, store
                o_sb = epi_pool.tile([D + 1, S], F32, tag="osb")
                nc.vector.tensor_copy(o_sb, out_ps)
                for qi in range(S // 128):
                    tr_ps = ps_st.tile([128, D + 1], F32, tag="st")
                    nc.tensor.transpose(
                        tr_ps,
                        o_sb[:, qi * 128 : (qi + 1) * 128],
                        identity[: D + 1, : D + 1],
                    )
                    recip = out_pool.tile([128, 1], F32, tag="recip")
                    nc.vector.reciprocal(recip, tr_ps[:, D : D + 1])
                    oq = out_pool.tile([128, D], F32, tag="oq")
                    nc.vector.tensor_scalar_mul(oq, tr_ps[:, 0:D], recip)
                    nc.sync.dma_start(
                        out=out_p[h, qi * 128 : (qi + 1) * 128, :], in_=oq
                    )
    if not nc.is_finalized():
        nc.finalize()
    return nc


def _prep_inputs(q, k, v, mask):
    """Host-side shard + gather + layout. Returns (in_maps, n_kp)."""
    keep = [np.flatnonzero(~mask[b, :, 0]) for b in range(B)]
    n_kp = max(128, -(-max(len(kb) for kb in keep) // 128) * 128)
    n_kt = n_kp // 128

    in_maps = []
    for c in range(N_CORES):
        b = c // 2
        h0 = (c % 2) * HPC
        kb = keep[b]
        nk = len(kb)

        qt = np.ascontiguousarray(q[b, h0 : h0 + HPC].transpose(0, 2, 1))

        kg = np.zeros((HPC, n_kp, D), np.float32)
        kg[:, :nk] = k[b, h0 : h0 + HPC][:, kb]
        kt = np.ascontiguousarray(kg.transpose(0, 2, 1))

        vg = np.zeros((HPC, n_kp, D + 1), np.float32)
        vg[:, :nk, :D] = v[b, h0 : h0 + HPC][:, kb]
        vg[:, :, D] = 1.0
        # [HPC, n_kt, 128, 65] -> [HPC, 128, n_kt, 65] (partition-major)
        vaug = np.ascontiguousarray(
            vg.reshape(HPC, n_kt, 128, D + 1).transpose(0, 2, 1, 3)
        )

        bias = np.zeros((128, n_kt), np.float32)
        idx = np.arange(n_kp).reshape(n_kt, 128).T  # [128, n_kt]
        bias[idx >= nk] = -100.0

        in_maps.append({"qt": qt, "kt": kt, "vaug": vaug, "bias": bias})
    return in_maps, n_kp


def kernel(q, k, v, mask, _trace=False):
    q = np.asarray(q, np.float32)
    k = np.asarray(k, np.float32)
    v = np.asarray(v, np.float32)
    mask = np.asarray(mask, bool)

    in_maps, n_kp = _prep_inputs(q, k, v, mask)
    if n_kp not in _kernel_cache:
        _kernel_cache[n_kp] = _build_kernel(n_kp)
    nc = _kernel_cache[n_kp]

    res = run_bass_kernel_spmd(nc, in_maps, list(range(N_CORES)), trace=_trace)
    out = np.empty((B, H, S, D), np.float32)
    for c in range(N_CORES):
        b = c // 2
        h0 = (c % 2) * HPC
        out[b, h0 : h0 + HPC] = res.results[c]["out"]
    if _trace:
        return out, res
    return out


if __name__ == "__main__":
    rng = np.random.default_rng(0)
    q = rng.standard_normal((B, H, S, D), np.float32)
    k = rng.standard_normal((B, H, S, D), np.float32)
    v = rng.standard_normal((B, H, S, D), np.float32)
    mask = rng.integers(0, 2, (B, S, 1)).astype(bool)
    out = kernel(q, k, v, mask)
    print("out", out.shape, out.dtype, float(np.abs(out).max()))
